# revision 3
# baseline (speedup 1.0000x reference)
"""Distributed multi-head attention kernel for Trainium2 (8 NeuronCores).

Problem: nn_Attention (B=2, N=2048, DIM=1024, HEADS=16, DIM_HEAD=64, f32).

Sharding: data-parallel over batch (2) x tensor-parallel over head groups (4).
Core cid handles batch b = cid // 4 and heads [4g, 4g+4) where g = cid % 4.
Each core computes a partial output y_g = attn_out(heads g) @ Wo[rows g]; the
host sums the 4 partials per batch and adds the bias (the gather step for
row-sharded Wo).

Device algorithm (per core), all matmuls bf16 with f32 PSUM accumulation:
  qT = (Wq_g * scale)^T @ x^T        [256, 2048]   (scale folded into Wq)
  kT = Wk_g^T @ x^T                  [256, 2048]
  v  = x @ Wv_g                      [2048, 256]
  per head h, per 512-wide query chunk, accumulated over 16 key tiles:
    sT   = kT_h^T-tile @ qT_h        [128 nk, 512 nq]  (scores transposed)
    p    = exp(sT) * binmaskT        (no max subtraction needed: |s| <~ 30)
    oT  += v_h-tile^T @ p            [64, 512]
    sum += ones^T @ p                [1, 512]
  outT_h = oT * broadcast(1/sum)     (K=1 outer-product matmul broadcast)
  y_g = outT^T @ Wo_g                [2048, 1024] f32
"""

import numpy as np
import ml_dtypes

B, N, DIM = 2, 2048, 1024
HEADS, DIM_HEAD = 16, 64
SCALE = DIM_HEAD ** -0.5
G = 4               # head groups (tensor-parallel degree)
HPG = HEADS // G    # heads per group = 4
INNER_G = HPG * DIM_HEAD  # 256 inner dims per group
N_CORES = 8
P = 128
NQ_CHUNK = 512
N_KT = N // P       # 16 key tiles
N_QC = N // NQ_CHUNK  # 4 query chunks
N_DT = DIM // P     # 8 dim tiles

bf16 = ml_dtypes.bfloat16

_cache = {}


def _build():
    import concourse.mybir as mybir
    import concourse.tile as tile
    from concourse import bacc

    f32 = mybir.dt.float32
    bf = mybir.dt.bfloat16
    Exp = mybir.ActivationFunctionType.Exp

    nc = bacc.Bacc("TRN2", target_bir_lowering=False, debug=False,
                   num_devices=N_CORES)

    xT_ext = nc.dram_tensor("xT", [DIM, N], bf, kind="ExternalInput")
    wq_ext = nc.dram_tensor("wq", [DIM, INNER_G], bf, kind="ExternalInput")
    wk_ext = nc.dram_tensor("wk", [DIM, INNER_G], bf, kind="ExternalInput")
    wv_ext = nc.dram_tensor("wv", [DIM, INNER_G], bf, kind="ExternalInput")
    wo_ext = nc.dram_tensor("wo", [INNER_G, DIM], bf, kind="ExternalInput")
    mk_ext = nc.dram_tensor("maskT", [N, N], bf, kind="ExternalInput")
    y_ext = nc.dram_tensor("y", [N, DIM], f32, kind="ExternalOutput")

    with tile.TileContext(nc) as tc:
        with (
            tc.tile_pool(name="persist", bufs=1) as persist,
            tc.tile_pool(name="pt_pool", bufs=6) as pt_pool,
            tc.tile_pool(name="tmp_pool", bufs=6) as tmp_pool,
            tc.tile_pool(name="ysb_pool", bufs=3) as ysb_pool,
            tc.tile_pool(name="small", bufs=4) as small,
            tc.tile_pool(name="ps_mm", bufs=2, space="PSUM") as ps_mm,
            tc.tile_pool(name="ps_s", bufs=2, space="PSUM") as ps_s,
            tc.tile_pool(name="ps_o", bufs=2, space="PSUM") as ps_o,
            tc.tile_pool(name="ps_sum", bufs=1, space="PSUM") as ps_sum,
            tc.tile_pool(name="ps_b", bufs=1, space="PSUM") as ps_b,
        ):
            # ---- resident SBUF tensors ----
            xt = persist.tile([P, N_DT, N], bf)          # x^T tiles
            mk = persist.tile([P, N_KT, N], bf)          # binary mask^T tiles
            wq = persist.tile([P, N_DT, INNER_G], bf)
            wk = persist.tile([P, N_DT, INNER_G], bf)
            wv = persist.tile([P, N_DT, INNER_G], bf)
            wo = persist.tile([P, INNER_G // P, DIM], bf)
            qT = persist.tile([P, 2, N], bf)             # [256, 2048] as 2 ptiles
            kT = persist.tile([P, 2, N], bf)
            vt = persist.tile([P, N_KT, HPG, DIM_HEAD], bf)  # v tiles per (kt, h)
            outT = persist.tile([P, 2, N], bf)           # normalized attn out^T
            ones = persist.tile([33, 64], bf)            # lhsT for bcast matmuls
            nc.vector.memset(ones[:], 1.0)
            ones128 = persist.tile([P, 1], bf)           # lhsT for sum matmuls
            nc.vector.memset(ones128[:], 1.0)

            # ---- input DMAs ----
            for dt_ in range(N_DT):
                nc.sync.dma_start(out=xt[:, dt_, :],
                                  in_=xT_ext.ap()[dt_ * P:(dt_ + 1) * P, :])
            for kt_ in range(N_KT):
                nc.sync.dma_start(out=mk[:, kt_, :],
                                  in_=mk_ext.ap()[kt_ * P:(kt_ + 1) * P, :])
            nc.sync.dma_start(
                out=wq[:], in_=wq_ext.ap().rearrange("(t p) m -> p t m", p=P))
            nc.sync.dma_start(
                out=wk[:], in_=wk_ext.ap().rearrange("(t p) m -> p t m", p=P))
            nc.sync.dma_start(
                out=wv[:], in_=wv_ext.ap().rearrange("(t p) m -> p t m", p=P))
            nc.sync.dma_start(
                out=wo[:], in_=wo_ext.ap().rearrange("(t p) m -> p t m", p=P))

            # ---- phase 1: Q/K/V projections ----
            # qT/kT: [256, 2048] = W^T @ x^T, lhsT = W tile, rhs = x^T tile
            for w_sb, dst in ((wq, qT), (wk, kT)):
                for pt_ in range(2):
                    for c in range(N // NQ_CHUNK):
                        acc = ps_mm.tile([P, NQ_CHUNK], f32, tag="mm512")
                        for dt_ in range(N_DT):
                            nc.tensor.matmul(
                                acc[:],
                                lhsT=w_sb[:, dt_, pt_ * P:(pt_ + 1) * P],
                                rhs=xt[:, dt_, c * NQ_CHUNK:(c + 1) * NQ_CHUNK],
                                start=(dt_ == 0), stop=(dt_ == N_DT - 1))
                        nc.scalar.activation(
                            out=dst[:, pt_, c * NQ_CHUNK:(c + 1) * NQ_CHUNK],
                            in_=acc[:], func=mybir.ActivationFunctionType.Copy)
            # v: [2048, 256] = x @ Wv, lhsT = x^T tile, rhs = Wv tile
            for kt_ in range(N_KT):
                acc = ps_mm.tile([P, INNER_G], f32, tag="mm512")
                for dt_ in range(N_DT):
                    nc.tensor.matmul(
                        acc[:, :INNER_G],
                        lhsT=xt[:, dt_, kt_ * P:(kt_ + 1) * P],
                        rhs=wv[:, dt_, :],
                        start=(dt_ == 0), stop=(dt_ == N_DT - 1))
                nc.vector.tensor_copy(
                    out=vt[:, kt_, :, :],
                    in_=acc[:, :INNER_G].rearrange("p (h d) -> p h d", h=HPG))

            # ---- phase 2: attention, heads processed in pairs ----
            for pair in range(2):          # heads (2*pair, 2*pair+1)
                for qc in range(N_QC):
                    cs = slice(qc * NQ_CHUNK, (qc + 1) * NQ_CHUNK)
                    o_acc = ps_o.tile([P, NQ_CHUNK], f32, tag="o")
                    s_acc = ps_sum.tile([33, NQ_CHUNK], f32, tag="sum")
                    for kt_ in range(N_KT):
                        ks = slice(kt_ * P, (kt_ + 1) * P)
                        for sub in range(2):   # head within pair
                            h = 2 * pair + sub
                            hp = slice(sub * 64, (sub + 1) * 64)
                            sc = ps_s.tile([P, NQ_CHUNK], f32, tag="s")
                            nc.tensor.matmul(
                                sc[:], lhsT=kT[hp, pair, ks],
                                rhs=qT[hp, pair, cs], start=True, stop=True)
                            pe = tmp_pool.tile([P, NQ_CHUNK], bf, tag="pe")
                            nc.scalar.activation(out=pe[:], in_=sc[:], func=Exp)
                            pt = pt_pool.tile([P, NQ_CHUNK], bf, tag="pt")
                            nc.vector.tensor_mul(pt[:], pe[:], mk[:, kt_, cs])
                            # attn @ v for this key tile (accumulating)
                            nc.tensor.matmul(
                                o_acc[sub * 64:(sub + 1) * 64, :],
                                lhsT=vt[:, kt_, h, :], rhs=pt[:],
                                start=(kt_ == 0), stop=(kt_ == N_KT - 1))
                            # softmax denominator (accumulating)
                            nc.tensor.matmul(
                                s_acc[sub * 32:sub * 32 + 1, :],
                                lhsT=ones128[:], rhs=pt[:],
                                start=(kt_ == 0), stop=(kt_ == N_KT - 1))
                    # normalize: outT_h = oT_h * (1/sum_h)
                    rec = small.tile([33, NQ_CHUNK], bf, tag="rec")
                    b_acc = ps_b.tile([P, NQ_CHUNK], f32, tag="b")
                    o_tmp = tmp_pool.tile([P, NQ_CHUNK], bf, tag="pe")
                    for sub in range(2):
                        rs = slice(sub * 32, sub * 32 + 1)
                        hp = slice(sub * 64, (sub + 1) * 64)
                        with nc.allow_low_precision(reason="softmax recip to bf16"):
                            nc.vector.reciprocal(out=rec[rs, :], in_=s_acc[rs, :])
                        nc.tensor.matmul(
                            b_acc[hp, :], lhsT=ones[rs.start:rs.start + 1, :],
                            rhs=rec[rs, :], start=True, stop=True)
                        nc.scalar.activation(
                            out=o_tmp[hp, :], in_=o_acc[hp, :],
                            func=mybir.ActivationFunctionType.Copy)
                        nc.vector.tensor_mul(
                            outT[hp, pair, cs], o_tmp[hp, :], b_acc[hp, :])

            # ---- phase 3: output projection y = outT^T @ Wo ----
            for mt in range(N // P):
                for ncn in range(DIM // NQ_CHUNK):
                    acc = ps_mm.tile([P, NQ_CHUNK], f32, tag="mm512")
                    for kt2 in range(INNER_G // P):
                        nc.tensor.matmul(
                            acc[:],
                            lhsT=outT[:, kt2, mt * P:(mt + 1) * P],
                            rhs=wo[:, kt2, ncn * NQ_CHUNK:(ncn + 1) * NQ_CHUNK],
                            start=(kt2 == 0), stop=(kt2 == INNER_G // P - 1))
                    y_sb = ysb_pool.tile([P, NQ_CHUNK], f32, tag="y")
                    nc.scalar.activation(
                        out=y_sb[:], in_=acc[:],
                        func=mybir.ActivationFunctionType.Copy)
                    nc.sync.dma_start(
                        out=y_ext.ap()[mt * P:(mt + 1) * P,
                                       ncn * NQ_CHUNK:(ncn + 1) * NQ_CHUNK],
                        in_=y_sb[:])

    nc.compile()
    return nc


def _get_nc():
    if "nc" not in _cache:
        _cache["nc"] = _build()
    return _cache["nc"]


def _prep_in_maps(x, mask, Wq, Wk, Wv, Wo):
    x = np.asarray(x, dtype=np.float32)
    mask = np.asarray(mask)
    xT = [np.ascontiguousarray(x[b].T).astype(bf16) for b in range(B)]
    mkT = [np.ascontiguousarray((mask[b, 0] == 0).T).astype(bf16)
           for b in range(B)]
    wqs = (np.asarray(Wq, np.float32) * SCALE).astype(bf16)
    wks = np.asarray(Wk, np.float32).astype(bf16)
    wvs = np.asarray(Wv, np.float32).astype(bf16)
    wos = np.asarray(Wo, np.float32).astype(bf16)
    in_maps = []
    for cid in range(N_CORES):
        b, g = cid // G, cid % G
        gs = slice(g * INNER_G, (g + 1) * INNER_G)
        in_maps.append({
            "xT": xT[b],
            "maskT": mkT[b],
            "wq": np.ascontiguousarray(wqs[:, gs]),
            "wk": np.ascontiguousarray(wks[:, gs]),
            "wv": np.ascontiguousarray(wvs[:, gs]),
            "wo": np.ascontiguousarray(wos[gs, :]),
        })
    return in_maps


def kernel(x, mask, Wq, Wk, Wv, Wo, bo):
    from concourse.bass_utils import run_bass_kernel_spmd

    nc = _get_nc()
    in_maps = _prep_in_maps(x, mask, Wq, Wk, Wv, Wo)
    res = run_bass_kernel_spmd(nc, in_maps, core_ids=list(range(N_CORES)))
    bo = np.asarray(bo, np.float32)
    y = np.empty((B, N, DIM), np.float32)
    for b in range(B):
        y[b] = res.results[b * G]["y"]
        for g in range(1, G):
            y[b] += res.results[b * G + g]["y"]
        y[b] += bo
    return y


# revision 8
# speedup vs baseline: 1.0182x; 1.0182x over previous
"""Distributed multi-head attention kernel for Trainium2 (8 NeuronCores).

Problem: nn_Attention (B=2, N=2048, DIM=1024, HEADS=16, DIM_HEAD=64, f32).

Sharding: data-parallel over batch (2) x tensor-parallel over head groups (4).
Core cid handles batch b = cid // 4 and heads [4g, 4g+4) where g = cid % 4.
Each core computes a partial output y_g = attn_out(heads g) @ Wo[rows g]; the
host sums the 4 partials per batch and adds the bias (the gather step for
row-sharded Wo).

Device algorithm (per core), all matmuls bf16 with f32 PSUM accumulation:
  qT = (Wq_g * scale)^T @ x^T        [256, 2048]   (scale folded into Wq)
  kT = Wk_g^T @ x^T                  [256, 2048]
  v  = x @ Wv_g                      [2048, 256]  (+ a ones column per head)
  per head h, per query chunk, accumulated over 16 key tiles:
    sT   = kT_h-tile @ qT_h          [128 nk, nq]  (scores transposed)
    p    = exp(sT) * binmaskT        (no max subtraction needed: |s| <~ 30)
    oT  += v_h-tile^T @ p            [65, nq]  (row 64 = softmax denominator)
  outT_h = oT * broadcast(1/oT[64])  (K=1 outer-product matmul broadcast)
  y_g = outT^T @ Wo_g                [2048, 1024] f32

Heads alternate base partition 0/64 so score matmuls (K=64) row-pack on the
PE array. exp/mask run on 1024-wide tiles (2 PSUM banks) to halve
elementwise op count. The output projection is interleaved per query chunk
to fill PE gaps and avoid a serial tail.
"""

import numpy as np
import ml_dtypes

B, N, DIM = 2, 2048, 1024
HEADS, DIM_HEAD = 16, 64
SCALE = DIM_HEAD ** -0.5
G = 4               # head groups (tensor-parallel degree)
HPG = HEADS // G    # heads per group = 4
INNER_G = HPG * DIM_HEAD  # 256 inner dims per group
N_CORES = 8
P = 128
NQ = 512            # PSUM-bank-sized matmul free dim
W = 1024            # elementwise tile width
N_KT = N // P       # 16 key tiles
N_DT = DIM // P     # 8 dim tiles

bf16 = ml_dtypes.bfloat16

_cache = {}


def _build():
    import concourse.mybir as mybir
    import concourse.tile as tile
    from concourse import bacc

    f32 = mybir.dt.float32
    bf = mybir.dt.bfloat16
    Exp = mybir.ActivationFunctionType.Exp
    Copy = mybir.ActivationFunctionType.Copy

    nc = bacc.Bacc("TRN2", target_bir_lowering=False, debug=False,
                   num_devices=N_CORES)

    xT_ext = nc.dram_tensor("xT", [DIM, N], bf, kind="ExternalInput")
    wq_ext = nc.dram_tensor("wq", [DIM, INNER_G], bf, kind="ExternalInput")
    wk_ext = nc.dram_tensor("wk", [DIM, INNER_G], bf, kind="ExternalInput")
    wv_ext = nc.dram_tensor("wv", [DIM, INNER_G], bf, kind="ExternalInput")
    wo_ext = nc.dram_tensor("wo", [INNER_G, DIM], bf, kind="ExternalInput")
    mk_ext = nc.dram_tensor("maskT", [N, N], bf, kind="ExternalInput")
    y_ext = nc.dram_tensor("y", [N, DIM], f32, kind="ExternalOutput")

    with tile.TileContext(nc) as tc:
        with (
            tc.tile_pool(name="persist", bufs=1) as persist,
            tc.tile_pool(name="pt_pool", bufs=6) as pt_pool,
            tc.tile_pool(name="tmp_pool", bufs=6) as tmp_pool,
            tc.tile_pool(name="ysb_pool", bufs=3) as ysb_pool,
            tc.tile_pool(name="small", bufs=4) as small,
            tc.tile_pool(name="ps_mm", bufs=2, space="PSUM") as ps_mm,
            tc.tile_pool(name="ps_s", bufs=2, space="PSUM") as ps_s,
            tc.tile_pool(name="ps_o", bufs=2, space="PSUM") as ps_o,
        ):
            # ---- resident SBUF tensors ----
            xt = persist.tile([P, N_DT, N], bf)          # x^T tiles
            mk = persist.tile([P, N_KT, N], bf)          # binary mask^T tiles
            wq = persist.tile([P, N_DT, INNER_G], bf)
            wk = persist.tile([P, N_DT, INNER_G], bf)
            wv = persist.tile([P, N_DT, INNER_G], bf)
            wo = persist.tile([P, INNER_G // P, DIM], bf)
            qT = persist.tile([P, 2, N], bf)             # [256, 2048], 2 ptiles
            kT = persist.tile([P, 2, N], bf)
            vt = persist.tile([P, N_KT, HPG, DIM_HEAD + 1], bf)
            outT = persist.tile([P, 2, N], bf)           # normalized attn out^T
            ones = persist.tile([P, 64], bf)             # lhsT for bcast matmuls
            nc.vector.memset(ones[:], 1.0)

            # ---- input DMAs (weights first: phase 1 needs them earliest) ----
            nc.sync.dma_start(
                out=wq[:], in_=wq_ext.ap().rearrange("(t p) m -> p t m", p=P))
            nc.sync.dma_start(
                out=wk[:], in_=wk_ext.ap().rearrange("(t p) m -> p t m", p=P))
            nc.sync.dma_start(
                out=wv[:], in_=wv_ext.ap().rearrange("(t p) m -> p t m", p=P))
            nc.sync.dma_start(
                out=wo[:], in_=wo_ext.ap().rearrange("(t p) m -> p t m", p=P))
            for dt_ in range(N_DT):
                nc.sync.dma_start(out=xt[:, dt_, :],
                                  in_=xT_ext.ap()[dt_ * P:(dt_ + 1) * P, :])
            for kt_ in range(N_KT):
                nc.sync.dma_start(out=mk[:, kt_, :],
                                  in_=mk_ext.ap()[kt_ * P:(kt_ + 1) * P, :])

            # ---- phase 1: Q/K/V projections ----
            # qT/kT: [256, 2048] = W^T @ x^T, lhsT = W tile, rhs = x^T tile
            for w_sb, dst in ((wq, qT), (wk, kT)):
                for pt_ in range(2):
                    for c in range(N // NQ):
                        acc = ps_mm.tile([P, NQ], f32, tag="mm512")
                        for dt_ in range(N_DT):
                            nc.tensor.matmul(
                                acc[:],
                                lhsT=w_sb[:, dt_, pt_ * P:(pt_ + 1) * P],
                                rhs=xt[:, dt_, c * NQ:(c + 1) * NQ],
                                start=(dt_ == 0), stop=(dt_ == N_DT - 1))
                        nc.vector.tensor_copy(
                            out=dst[:, pt_, c * NQ:(c + 1) * NQ], in_=acc[:])
            # v: [2048, 256] = x @ Wv, lhsT = x^T tile, rhs = Wv tile
            for kt_ in range(N_KT):
                acc = ps_mm.tile([P, NQ], f32, tag="mm512")
                for dt_ in range(N_DT):
                    nc.tensor.matmul(
                        acc[:, :INNER_G],
                        lhsT=xt[:, dt_, kt_ * P:(kt_ + 1) * P],
                        rhs=wv[:, dt_, :],
                        start=(dt_ == 0), stop=(dt_ == N_DT - 1))
                nc.vector.memset(vt[:, kt_, :, DIM_HEAD:DIM_HEAD + 1], 1.0)
                nc.vector.tensor_copy(
                    out=vt[:, kt_, :, :DIM_HEAD],
                    in_=acc[:, :INNER_G].rearrange("p (h d) -> p h d", h=HPG))

            # ---- phases 2+3: attention + output projection per query chunk --
            for qc in range(N // W):      # 2 chunks of 1024 queries
                for h in range(HPG):
                    pt_i = h // 2
                    hp = slice((h % 2) * 64, (h % 2) * 64 + 64)
                    cs = slice(qc * W, (qc + 1) * W)
                    o_acc_a = ps_o.tile([65, NQ], f32, tag="o")
                    o_acc_b = ps_o.tile([65, NQ], f32, tag="o")
                    for kt_ in range(N_KT):
                        ks = slice(kt_ * P, (kt_ + 1) * P)
                        sc = ps_s.tile([P, W], f32, tag="s")
                        nc.tensor.matmul(
                            sc[:, :NQ], lhsT=kT[hp, pt_i, ks],
                            rhs=qT[hp, pt_i, qc * W:qc * W + NQ],
                            start=True, stop=True)
                        nc.tensor.matmul(
                            sc[:, NQ:], lhsT=kT[hp, pt_i, ks],
                            rhs=qT[hp, pt_i, qc * W + NQ:(qc + 1) * W],
                            start=True, stop=True)
                        pe = tmp_pool.tile([P, W], bf, tag="pe")
                        nc.scalar.activation(out=pe[:], in_=sc[:], func=Exp)
                        pt = pt_pool.tile([P, W], bf, tag="pt")
                        nc.vector.tensor_mul(pt[:], pe[:], mk[:, kt_, cs])
                        # attn @ v (+ denominator in row 64), accumulating
                        nc.tensor.matmul(
                            o_acc_a[:], lhsT=vt[:, kt_, h, :], rhs=pt[:, :NQ],
                            start=(kt_ == 0), stop=(kt_ == N_KT - 1))
                        nc.tensor.matmul(
                            o_acc_b[:], lhsT=vt[:, kt_, h, :], rhs=pt[:, NQ:],
                            start=(kt_ == 0), stop=(kt_ == N_KT - 1))
                    # normalize: outT_h = oT_h * bcast(1/sum_h)
                    for half, o_acc in ((0, o_acc_a), (1, o_acc_b)):
                        cs2 = slice(qc * W + half * NQ, qc * W + (half + 1) * NQ)
                        rec = small.tile([P, NQ], bf, tag="rec")
                        b_acc = ps_mm.tile([P, NQ], f32, tag="mm512")
                        o_tmp = tmp_pool.tile([P, W], bf, tag="pe")
                        with nc.allow_low_precision(reason="softmax recip bf16"):
                            nc.vector.reciprocal(out=rec[64:65, :],
                                                 in_=o_acc[64:65, :])
                        nc.tensor.matmul(
                            b_acc[hp, :], lhsT=ones[64:65, :],
                            rhs=rec[64:65, :], start=True, stop=True)
                        nc.scalar.activation(
                            out=o_tmp[hp, :NQ], in_=o_acc[0:64, :], func=Copy)
                        nc.vector.tensor_mul(
                            outT[hp, pt_i, cs2], o_tmp[hp, :NQ], b_acc[hp, :])
                # output projection for the tokens of this chunk
                for mt in range(qc * (W // P), (qc + 1) * (W // P)):
                    for ncn in range(DIM // NQ):
                        acc = ps_mm.tile([P, NQ], f32, tag="mm512")
                        for kt2 in range(INNER_G // P):
                            nc.tensor.matmul(
                                acc[:],
                                lhsT=outT[:, kt2, mt * P:(mt + 1) * P],
                                rhs=wo[:, kt2, ncn * NQ:(ncn + 1) * NQ],
                                start=(kt2 == 0), stop=(kt2 == INNER_G // P - 1))
                        y_sb = ysb_pool.tile([P, NQ], f32, tag="y")
                        nc.vector.tensor_copy(out=y_sb[:], in_=acc[:])
                        nc.sync.dma_start(
                            out=y_ext.ap()[mt * P:(mt + 1) * P,
                                           ncn * NQ:(ncn + 1) * NQ],
                            in_=y_sb[:])

    nc.compile()
    return nc


def _get_nc():
    if "nc" not in _cache:
        _cache["nc"] = _build()
    return _cache["nc"]


def _prep_in_maps(x, mask, Wq, Wk, Wv, Wo):
    x = np.asarray(x, dtype=np.float32)
    mask = np.asarray(mask)
    xT = [np.ascontiguousarray(x[b].T).astype(bf16) for b in range(B)]
    mkT = [np.ascontiguousarray((mask[b, 0] == 0).T).astype(bf16)
           for b in range(B)]
    wqs = (np.asarray(Wq, np.float32) * SCALE).astype(bf16)
    wks = np.asarray(Wk, np.float32).astype(bf16)
    wvs = np.asarray(Wv, np.float32).astype(bf16)
    wos = np.asarray(Wo, np.float32).astype(bf16)
    in_maps = []
    for cid in range(N_CORES):
        b, g = cid // G, cid % G
        gs = slice(g * INNER_G, (g + 1) * INNER_G)
        in_maps.append({
            "xT": xT[b],
            "maskT": mkT[b],
            "wq": np.ascontiguousarray(wqs[:, gs]),
            "wk": np.ascontiguousarray(wks[:, gs]),
            "wv": np.ascontiguousarray(wvs[:, gs]),
            "wo": np.ascontiguousarray(wos[gs, :]),
        })
    return in_maps


def kernel(x, mask, Wq, Wk, Wv, Wo, bo):
    from concourse.bass_utils import run_bass_kernel_spmd

    nc = _get_nc()
    in_maps = _prep_in_maps(x, mask, Wq, Wk, Wv, Wo)
    res = run_bass_kernel_spmd(nc, in_maps, core_ids=list(range(N_CORES)))
    bo = np.asarray(bo, np.float32)
    y = np.empty((B, N, DIM), np.float32)
    for b in range(B):
        y[b] = res.results[b * G]["y"]
        for g in range(1, G):
            y[b] += res.results[b * G + g]["y"]
        y[b] += bo
    return y
